# revision 20
# baseline (speedup 1.0000x reference)
"""ContextRetentionLayer Trainium2 kernel (mixed fp8/bf16).

Reference computation (per token t, d=1024, W=512 memory slots):
    s[t, w]   = (x[t] . mb[w]) / 32
    attn[t]   = softmax_w(s[t])
    r[t]      = sum_w attn[t, w] * mb[w]
    g[t]      = sigmoid(x[t] @ gw.T + gb)
    out[t]    = g[t] * x[t] + (1 - g[t]) * r[t]

Sharding: 4x4096 = 16384 tokens split evenly across 8 cores (2048 each);
memory_bank / gate weights replicated. Device layout is fully transposed
(d on partitions, tokens on the free axis); all weights are quantized and
pre-transposed on the host, so the device does no transposes.

Precision plan (PE is the roofline; fp8e4 + DoubleRow = 2 contraction rows
per cycle, measured ~1.44x over bf16 at N=512):
  scores     fp8e4 DoubleRow (x, mbT both e4m3)
  softmax    exp -> bf16 scaled by 64 (bias=ln64 folded into the activation);
             denominator via a (1/64)-column ones-matmul, so den comes out
             unscaled; normalize on DVE -> fp8 attn*64
  retrieved  fp8e4 DoubleRow (attn*64 @ mb_e4m3) -> PSUM holds 64*r
  gate       contraction split: chunks 0-3 fp8-DR, chunks 4-7 bf16 (both
             pre-scaled by 128 to keep e4m3 out of subnormals; sigmoid
             rescales with scale=1/128). Full-fp8 gate fails the 2e-2
             accuracy gate (2.1e-2); the 4/4 split sims at 1.36e-2.
  combine    t1 = (g*(-1/64)+1/64) * (64 r)  [affine_mul_reduce, one DVE op]
             t2 = g * x_bf16; out = t1 + t2  -> bf16 store
Skipping softmax max-subtraction is safe: scores/32 ~ N(0,1).

Per-core budget: PE ~78us (64+64+64 DR-MM + 144 bf16-MM @ N=512),
DVE ~60us, ScalarE ~35us, DMA ~14MB @ ~358GB/s ~40us.
"""

import math

import ml_dtypes
import numpy as np

import concourse.bass as bass
import concourse.tile as tile
from concourse import bacc, bass_utils, mybir
from concourse.bass import ts

AF = mybir.ActivationFunctionType
F32 = mybir.dt.float32
BF16 = mybir.dt.bfloat16
F8 = mybir.dt.float8e4
NP_F8 = ml_dtypes.float8_e4m3
NP_BF16 = ml_dtypes.bfloat16
DR = mybir.MatmulPerfMode.DoubleRow

N_CORES = 8
B, S, D = 4, 4096, 1024
W = 512
T_CORE = (B * S) // N_CORES  # 2048 tokens per core
T_TILE = 512                 # moving free dim per matmul
NT = T_CORE // T_TILE        # 4 token tiles
DC = D // 128                # 8 chunks of the embed dim
WC = W // 128                # 4 chunks of the memory window
NF8 = 4                      # gate contraction chunks done in fp8-DR (of DC)
GW_SCALE = 128.0             # gate weight prescale (keeps e4m3 normal)
A_SCALE = 64.0               # attn prescale into fp8


def _body(tc: tile.TileContext, reps: int = 1):
    nc = tc.nc

    xq8 = nc.dram_tensor("xq8", (D, T_CORE), F8, kind="ExternalInput").ap()
    xb = nc.dram_tensor("xb", (D, T_CORE), BF16, kind="ExternalInput").ap()
    mbt8 = nc.dram_tensor("mbt8", (D, W), F8, kind="ExternalInput").ap()
    mb8 = nc.dram_tensor("mb8", (W, D), F8, kind="ExternalInput").ap()
    gw8 = nc.dram_tensor("gw8", (D, D), F8, kind="ExternalInput").ap()
    gwb = nc.dram_tensor("gwb", (D, D), BF16, kind="ExternalInput").ap()
    gb = nc.dram_tensor("gb", (D,), F32, kind="ExternalInput").ap()
    id64 = nc.dram_tensor("id64", (128, 128), BF16, kind="ExternalInput").ap()
    outT = nc.dram_tensor("outt", (D, T_CORE), BF16, kind="ExternalOutput").ap()

    for _rep in range(reps):
        _emit_once(tc, xq8, xb, mbt8, mb8, gw8, gwb, gb, id64, outT)


def _emit_once(tc, xq8, xb, mbt8, mb8, gw8, gwb, gb, id64, outT):
    nc = tc.nc
    with (
        tc.tile_pool(name="const", bufs=1) as const,
        tc.tile_pool(name="big", bufs=1) as big,
        tc.tile_pool(name="work", bufs=3) as work,
        tc.tile_pool(name="mm_ps", bufs=6, space="PSUM") as mm_ps,
        tc.tile_pool(name="den_psp", bufs=1, space="PSUM") as den_psp,
    ):
        mbT_s = const.tile([128, DC, W], F8)
        mb_s = const.tile([128, WC, D], F8)
        gw8_s = const.tile([128, NF8, D], F8)
        gwb_s = const.tile([128, DC - NF8, D], BF16)
        gb_s = const.tile([128, DC], F32)
        # [128, 2, 16] so the DR k-pair stride is 16B (s3_lw_dual_fp8 rule);
        # only column 0 is used as the lhsT.
        ones8_s = const.tile([128, 2, 16], F8)
        nc.vector.memset(ones8_s, 1.0)
        ln64_s = const.tile([128, 1], F32)
        nc.vector.memset(ln64_s, math.log(A_SCALE))
        id64_s = const.tile([128, 128], BF16)
        nc.sync.dma_start(out=id64_s, in_=id64)
        xq_s = big.tile([128, DC, T_CORE], F8)
        xb_s = big.tile([128, DC, T_CORE], BF16)

        mbTv = mbt8.rearrange("(c p) w -> p c w", p=128)
        mbv = mb8.rearrange("(c p) d -> p c d", p=128)
        gw8v = gw8.rearrange("(c p) e -> p c e", p=128)
        gwbv = gwb.rearrange("(c p) e -> p c e", p=128)
        xqv = xq8.rearrange("(c p) t -> p c t", p=128)
        xbv = xb.rearrange("(c p) t -> p c t", p=128)

        # need-ordered loads: pass-1 deps first (mbT + x tile 0), then the
        # rest of x, then pass-2 weights streaming in behind.
        for dc in range(DC):
            nc.sync.dma_start(out=mbT_s[:, dc, :], in_=mbTv[:, dc, :])
        nc.sync.dma_start(out=xq_s[:, :, ts(0, T_TILE)], in_=xqv[:, :, ts(0, T_TILE)])
        nc.sync.dma_start(out=gb_s, in_=gb.rearrange("(c p) -> p c", p=128))
        for ti in range(1, NT):
            nc.sync.dma_start(
                out=xq_s[:, :, ts(ti, T_TILE)], in_=xqv[:, :, ts(ti, T_TILE)]
            )
        for wc in range(WC):
            nc.sync.dma_start(out=mb_s[:, wc, :], in_=mbv[:, wc, :])
        for kc in range(NF8):
            nc.sync.dma_start(out=gw8_s[:, kc, :], in_=gw8v[:, kc, :])
        for kc in range(DC - NF8):
            nc.sync.dma_start(out=gwb_s[:, kc, :], in_=gwbv[:, NF8 + kc, :])
        for ti in range(NT):
            nc.sync.dma_start(
                out=xb_s[:, :, ts(ti, T_TILE)], in_=xbv[:, :, ts(ti, T_TILE)]
            )

        e64_s = big.tile([128, WC, T_CORE], BF16)  # 64*exp(s/32)
        e8_s = big.tile([128, WC, T_CORE], F8)     # exp(s/32) in e4m3 (den path)
        at8_s = big.tile([128, WC, T_CORE], F8)    # attn*64 in e4m3
        rd_s = big.tile([1, T_CORE], F32)          # 1 / denominator
        rb_s = big.tile([128, T_CORE], F32)        # broadcast across partitions

        # ---- pass 1: scores (fp8-DR), exp, denominators, attn normalize
        for ti in range(NT):
            tsl = ts(ti, T_TILE)
            den_ps = den_psp.tile([1, T_TILE], F32, tag="den")
            for wc in range(WC):
                s_ps = mm_ps.tile([128, T_TILE], F32, tag="mm")
                for kp in range(0, DC, 2):
                    nc.tensor.matmul(
                        s_ps,
                        lhsT=mbT_s[:, kp : kp + 2, ts(wc, 128)],
                        rhs=xq_s[:, kp : kp + 2, tsl],
                        start=(kp == 0),
                        stop=(kp == DC - 2),
                        perf_mode=DR,
                    )
                # e64 = exp(s/32 + ln 64) = 64*exp(s/32)
                nc.scalar.activation(
                    out=e64_s[:, wc, tsl], in_=s_ps, func=AF.Exp,
                    scale=1.0 / 32.0, bias=ln64_s[:, 0:1],
                )
                # e8 = exp(s/32) for the fp8-DR denominator matmul
                nc.scalar.activation(
                    out=e8_s[:, wc, tsl], in_=s_ps, func=AF.Exp,
                    scale=1.0 / 32.0,
                )
            for wp in range(0, WC, 2):
                nc.tensor.matmul(
                    den_ps,
                    lhsT=ones8_s[:, :, 0:1],
                    rhs=e8_s[:, wp : wp + 2, tsl],
                    start=(wp == 0),
                    stop=(wp == WC - 2),
                    perf_mode=DR,
                )
            rscr = work.tile([1, T_TILE], F32, tag="rscr")
            nc.vector.reciprocal_approx_accurate(
                out=rd_s[:, tsl], in_=den_ps, scratch=rscr
            )
            nc.gpsimd.partition_broadcast(rb_s[:, tsl], rd_s[:, tsl])
            for wc in range(WC):
                nc.vector.tensor_mul(
                    at8_s[:, wc, tsl], e64_s[:, wc, tsl], rb_s[:, tsl]
                )

        # ---- pass 2: gate (fp8-DR + bf16 split), retrieved (fp8-DR), combine
        outv = outT.rearrange("(c p) t -> p c t", p=128)

        def p2_iter(dc, t0, tw):
            tsl = slice(t0, t0 + tw)
            z_ps = mm_ps.tile([128, tw], F32, tag="mm")
            for kp in range(0, NF8, 2):
                nc.tensor.matmul(
                    z_ps,
                    lhsT=gw8_s[:, kp : kp + 2, ts(dc, 128)],
                    rhs=xq_s[:, kp : kp + 2, tsl],
                    start=(kp == 0),
                    stop=False,
                    perf_mode=DR,
                )
            for kc in range(DC - NF8):
                nc.tensor.matmul(
                    z_ps,
                    lhsT=gwb_s[:, kc, ts(dc, 128)],
                    rhs=xb_s[:, NF8 + kc, tsl],
                    start=False,
                    stop=(kc == DC - NF8 - 1),
                )
            g = work.tile([128, tw], BF16, tag="g")
            nc.scalar.activation(
                out=g, in_=z_ps, func=AF.Sigmoid,
                scale=1.0 / GW_SCALE, bias=gb_s[:, dc : dc + 1],
            )
            # PSUM = 64*x - 64*r  (mb8 is host-negated; id64 = 64*I)
            r_ps = mm_ps.tile([128, tw], F32, tag="mm")
            nc.tensor.matmul(
                r_ps,
                lhsT=id64_s[:, ts(0, 128)],
                rhs=xb_s[:, dc, tsl],
                start=True,
                stop=False,
            )
            for wp in range(0, WC, 2):
                nc.tensor.matmul(
                    r_ps,
                    lhsT=mb_s[:, wp : wp + 2, ts(dc, 128)],
                    rhs=at8_s[:, wp : wp + 2, tsl],
                    start=False,
                    stop=(wp == WC - 2),
                    perf_mode=DR,
                )
            # t1 = (g*(-1/64) + 1/64) * 64*(x-r) = (1-g)*(x-r)
            t1 = work.tile([128, tw], BF16, tag="t1")
            acc = work.tile([128, 1], F32, tag="acc")
            nc.vector.affine_mul_reduce(
                out=t1, accum_out=acc, in0=g, in1=r_ps,
                scale=-1.0 / A_SCALE, bias=1.0 / A_SCALE,
            )
            o = work.tile([128, tw], BF16, tag="o")
            nc.vector.tensor_sub(o, xb_s[:, dc, tsl], t1)
            nc.sync.dma_start(out=outv[:, dc, tsl], in_=o)

        for ti in range(NT):
            for dc in range(DC):
                if ti == NT - 1 and dc == DC - 1:
                    p2_iter(dc, ti * T_TILE, T_TILE // 2)
                    p2_iter(dc, ti * T_TILE + T_TILE // 2, T_TILE // 2)
                else:
                    p2_iter(dc, ti * T_TILE, T_TILE)


_NC_CACHE = None


def _build_nc(reps: int = 1):
    global _NC_CACHE
    if reps == 1 and _NC_CACHE is not None:
        return _NC_CACHE
    nc = bacc.Bacc("TRN2", target_bir_lowering=False, debug=False,
                   enable_asserts=False)
    with tile.TileContext(nc) as tc:
        _body(tc, reps)
    nc.compile()
    if reps == 1:
        _NC_CACHE = nc
    return nc


def make_in_maps(x, memory_bank, gate_w, gate_b):
    x = np.ascontiguousarray(np.asarray(x, np.float32)).reshape(B * S, D)
    mb_n = np.asarray(memory_bank, np.float32)
    gwT128 = np.asarray(gate_w, np.float32).T * GW_SCALE
    mbt8_n = np.ascontiguousarray(mb_n.T.astype(NP_F8))
    mb8_n = np.ascontiguousarray((-mb_n).astype(NP_F8))
    id64_n = np.ascontiguousarray((A_SCALE * np.eye(128)).astype(NP_BF16))
    gw8_n = np.ascontiguousarray(gwT128.astype(NP_F8))
    gwb_n = np.ascontiguousarray(gwT128.astype(NP_BF16))
    gb_n = np.ascontiguousarray(np.asarray(gate_b, np.float32))
    in_maps = []
    for c in range(N_CORES):
        xsT = np.ascontiguousarray(x[c * T_CORE : (c + 1) * T_CORE].T)
        in_maps.append(
            {
                "xq8": xsT.astype(NP_F8),
                "xb": xsT.astype(NP_BF16),
                "mbt8": mbt8_n,
                "mb8": mb8_n,
                "gw8": gw8_n,
                "gwb": gwb_n,
                "gb": gb_n,
                "id64": id64_n,
            }
        )
    return in_maps


def assemble_out(results):
    shards = [results[c]["outt"].astype(np.float32).T for c in range(N_CORES)]
    return np.concatenate(shards, axis=0).reshape(B, S, D)


def kernel(x, memory_bank, gate_w, gate_b, _run_kwargs=None):
    nc = _build_nc()
    in_maps = make_in_maps(x, memory_bank, gate_w, gate_b)
    res = bass_utils.run_bass_kernel_spmd(
        nc, in_maps, core_ids=list(range(N_CORES)), **(_run_kwargs or {})
    )
    out = assemble_out(res.results)
    if _run_kwargs:
        kernel.last_result = res
    return out
